# revision 5
# baseline (speedup 1.0000x reference)
"""Mixtral-style MoE (top-2 of 8 experts) on 8 TRN2 NeuronCores.

Strategy (expert-parallel, matching TENSOR_EXPERT_PARALLEL):
  - Host: router (logits -> softmax -> top-2 -> normalized weights), then
    shard: core e receives up to C=1024 tokens routed to expert e
    (gathered and pre-transposed to [H, C]) plus expert e's w1/w3/w2
    (bf16, pre-packed into PE-friendly [128 x free] tiles).  C=1024 is
    chosen so every core does identical, full-partition work (8 chunks
    of 128 tokens); the few overflow pairs beyond 1024 per expert
    (~1.3% of the 8192 token-expert pairs at balanced routing) are
    computed exactly on the host in fp32 and added into the output.
  - Device (SPMD, identical program on 8 cores): h1T = w1 @ xeT,
    h3T = w3 @ xeT, gT = silu(h1T) * h3T (bf16), outT = gT.T @ w2T,
    scaled per-token by the routing weight.  Pure GEMM pipeline; all
    DMAs are fully linear.
  - Host: scatter-add each core's [count_e, H] contribution into the
    [T, H] output (each token appears in exactly TOP_K=2 expert lists).

Compute is done in bf16 (fp32 accumulation in PSUM), which keeps the
TensorEngine at its 78.6 TF/s peak; sparse routing means each core does
C = 1024 token-columns instead of all 4096 (4x fewer FLOPs than dense).

v2 layout/startup notes (from baseline trace analysis):
  - The real-matmul stream is issue-dense at 216 ns per 512-col bf16
    matmul (the PE roofline); all remaining overhead is at the edges:
    ~12.3 us before the first real matmul (6.5 us fixed BSP preamble +
    DMA issue serialization + an 11x512-col warmup run at the HAM
    half-clock), ~1.5 us of PSUM-pool-transition stall between the
    up-proj and down-proj phases, and ~5.5 us of drain + teardown.
  - v2 therefore: (1) spreads the startup-critical DMAs across the
    Sync/GpSimd/Vector queues (Scalar starts with a compiler-hoisted
    1.3 us ACT_TABLE_LOAD, so it only gets non-critical loads),
    (2) uses short 128-col warmups that bridge only until the first
    real operands land (~1 us) letting the real stream ride the HAM
    ramp instead of waiting for it, (3) runs phase B in the SAME PSUM
    pool/tags as phase A so there is no pool barrier at the A->B
    transition, (4) fuses the per-panel DMAs (w1||w3 paired per fp,
    w2 in 4 blocks, xe in 5 slices of one [P, HK*C] tile, cv packed
    [P, CK]) cutting DMA issues from ~122 to ~58, and (5) finishes on
    two 256-col down-proj chains so the end-of-kernel drain is short.
"""

import numpy as np
import ml_dtypes

B, S, H, F, E, TOP_K = 2, 2048, 1024, 3584, 8, 2
N_CORES = 8
P = 128
HK = H // P   # 8 contraction chunks for up-proj
FP = F // P   # 28 partition chunks of the FFN dim
C = 1024      # per-core token capacity (8 full 128-token chunks)
CK = C // P
NW2 = 4       # w2 arrives in 4 fused blocks of 7 panels
W2B = FP // NW2
HQ = 2 * P    # startup-critical first columns of the fp0/fp1 panels
NWARM = 8     # 128-col HAM warmup matmuls (bridge until operands land)

BF16 = ml_dtypes.bfloat16

_BUILD_CACHE = {}
LAST_EXEC_TIME_NS = None


def _ensure_axon_hooks_stub():
    """bass_utils imports antenv.axon_hooks when BASS_TRACE is set; the
    agent image lacks it.  Register a None-hook stub so a stray
    BASS_TRACE env var degrades to an untraced run instead of crashing.
    """
    import sys, types

    try:
        import antenv.axon_hooks  # noqa: F401
        return
    except ImportError:
        pass
    mod = types.ModuleType("antenv.axon_hooks")
    mod._hook = None
    mod.set_axon_ntff_profile_hook = lambda h: setattr(mod, "_hook", h)
    mod.get_axon_ntff_profile_hook = lambda: mod._hook
    sys.modules["antenv.axon_hooks"] = mod
    try:
        import antenv

        antenv.axon_hooks = mod
    except ImportError:
        pass


def _build():
    """Build + compile the SPMD Bass program (token capacity C=1024)."""
    import concourse.bacc as bacc
    import concourse.mybir as mybir
    from concourse.tile import TileContext

    bf = mybir.dt.bfloat16
    f32 = mybir.dt.float32

    nc = bacc.Bacc("TRN2", target_bir_lowering=False, debug=False,
                   num_devices=N_CORES)
    xep = nc.dram_tensor("xep", [P, HK * C], bf, kind="ExternalInput")
    wpk = nc.dram_tensor("wpk", [FP, P, 2 * H], bf, kind="ExternalInput")
    w2f = nc.dram_tensor("w2f", [NW2, P, W2B * H], bf, kind="ExternalInput")
    cvp = nc.dram_tensor("cvp", [P, CK], f32, kind="ExternalInput")
    out = nc.dram_tensor("out", [C, H], bf, kind="ExternalOutput")

    cn_chunks = [(0, 512), (512, 512)]
    silu = mybir.ActivationFunctionType.Silu
    copy = mybir.ActivationFunctionType.Copy

    with TileContext(nc) as tc:
        with (
            tc.tile_pool(name="persist", bufs=1) as persist,
            tc.tile_pool(name="wload", bufs=3) as wload,
            tc.tile_pool(name="evac", bufs=4) as evac,
            tc.tile_pool(name="ost", bufs=3) as ost,
            tc.tile_pool(name="ps", bufs=4, space="PSUM") as ps,
        ):
            warm = persist.tile([P, 512], bf, tag="warm", name="warm")
            xet = persist.tile([P, HK * C], bf, tag="xe", name="xet")
            wt0 = persist.tile([P, 2 * H], bf, tag="wt0", name="wt0")
            wt1 = persist.tile([P, 2 * H], bf, tag="wt1", name="wt1")
            gt = [persist.tile([P, C], bf, tag=f"g{fp}", name=f"g{fp}")
                  for fp in range(FP)]
            w2s = [persist.tile([P, W2B * H], bf, tag=f"w2_{i}",
                                name=f"w2_{i}") for i in range(NW2)]
            cvt = persist.tile([P, CK], f32, tag="cv", name="cvt")

            # Startup: the warmup memset is Vector's first op; the
            # critical operand set (fp0/fp1 panel heads + xe chunk 0)
            # is spread over the Sync/GpSimd/Vector issue queues so the
            # first real matmul group unblocks ~1 us after the engines
            # come up.  Scalar opens with the compiler-hoisted
            # ACT_TABLE_LOAD (~1.3 us), so it only gets loads that are
            # not needed until the hk>=2 part of the lockstep.
            nc.vector.memset(warm[:], 0.0)
            nc.sync.dma_start(out=wt0[:, 0:HQ], in_=wpk[0][:, 0:HQ])
            nc.gpsimd.dma_start(out=wt0[:, H:H + HQ], in_=wpk[0][:, H:H + HQ])
            nc.sync.dma_start(out=xet[:, 0:512], in_=xep[:, 0:512])
            nc.gpsimd.dma_start(out=wt1[:, 0:HQ], in_=wpk[1][:, 0:HQ])
            nc.gpsimd.dma_start(out=wt1[:, H:H + HQ], in_=wpk[1][:, H:H + HQ])
            nc.sync.dma_start(out=xet[:, 512:C], in_=xep[:, 512:C])
            nc.gpsimd.dma_start(out=xet[:, C:2 * C], in_=xep[:, C:2 * C])
            nc.sync.dma_start(out=wt0[:, HQ:H], in_=wpk[0][:, HQ:H])
            nc.scalar.dma_start(out=wt0[:, H + HQ:2 * H],
                                in_=wpk[0][:, H + HQ:2 * H])
            nc.scalar.dma_start(out=wt1[:, HQ:H], in_=wpk[1][:, HQ:H])
            nc.scalar.dma_start(out=wt1[:, H + HQ:2 * H],
                                in_=wpk[1][:, H + HQ:2 * H])
            nc.gpsimd.dma_start(out=xet[:, 2 * C:3 * C], in_=xep[:, 2 * C:3 * C])
            nc.gpsimd.dma_start(out=xet[:, 3 * C:HK * C], in_=xep[:, 3 * C:HK * C])

            # Short 128-col warmups: keep the PE busy (HAM ramp) for the
            # ~1 us until the first real operands land; the real stream
            # then continues the ramp doing useful work.
            wps = ps.tile([P, 512], f32, tag="ps1", name="wps")
            for i in range(NWARM):
                nc.tensor.matmul(wps[:, 0:P], warm[:, 0:P], warm[:, 0:P],
                                 start=True, stop=True)

            # Phase A: h1T/h3T = w1/w3 @ xeT per 128-row chunk of F,
            # fused SwiGLU into gT (bf16).  fp0 + fp1 run in hk-LOCKSTEP
            # so the startup DMA stream stays ahead of the PE.
            pss = {}
            for fp in (0, 1):
                for mat in (1, 3):
                    for ci in range(len(cn_chunks)):
                        pss[(fp, mat, ci)] = ps.tile(
                            [P, 512], f32, tag=f"ps{mat}",
                            name=f"ps{mat}_f{fp}_c{ci}",
                        )
            wts = {0: wt0, 1: wt1}
            for hk in range(HK):
                for ci, (coff, csz) in enumerate(cn_chunks):
                    for fp in (0, 1):
                        for mat in (1, 3):
                            off = 0 if mat == 1 else H
                            wt = wts[fp]
                            nc.tensor.matmul(
                                pss[(fp, mat, ci)][:, :csz],
                                wt[:, off + hk * P:off + (hk + 1) * P],
                                xet[:, hk * C + coff:hk * C + coff + csz],
                                start=(hk == 0), stop=(hk == HK - 1),
                            )
            for fp in (0, 1):
                for ci, (coff, csz) in enumerate(cn_chunks):
                    sil = evac.tile([P, 512], f32, tag="sil",
                                    name=f"sil_f{fp}_{ci}")
                    nc.scalar.activation(
                        sil[:, :csz], pss[(fp, 1, ci)][:, :csz], silu)
                    nc.vector.tensor_mul(
                        gt[fp][:, coff:coff + csz], sil[:, :csz],
                        pss[(fp, 3, ci)][:, :csz],
                    )
            for fp in range(2, FP):
                wt = wload.tile([P, 2 * H], bf, tag="wp")
                nc.sync.dma_start(out=wt[:], in_=wpk[fp])
                # Phase-B operands ride the idle GpSimd queue mid-phase-A
                # so their transfers overlap the A tail without delaying
                # the panel stream.
                if fp == 14:
                    nc.gpsimd.dma_start(out=cvt[:], in_=cvp[:, :])
                if fp in (16, 18, 20, 22):
                    i = (fp - 16) // 2
                    nc.gpsimd.dma_start(out=w2s[i][:], in_=w2f[i])
                for ci, (coff, csz) in enumerate(cn_chunks):
                    ps1 = ps.tile([P, 512], f32, tag="ps1")
                    ps3 = ps.tile([P, 512], f32, tag="ps3")
                    for hk in range(HK):
                        nc.tensor.matmul(
                            ps1[:, :csz],
                            wt[:, hk * P:(hk + 1) * P],
                            xet[:, hk * C + coff:hk * C + coff + csz],
                            start=(hk == 0), stop=(hk == HK - 1),
                        )
                    sil = evac.tile([P, 512], f32, tag="sil")
                    for hk in range(HK):
                        nc.tensor.matmul(
                            ps3[:, :csz],
                            wt[:, H + hk * P:H + (hk + 1) * P],
                            xet[:, hk * C + coff:hk * C + coff + csz],
                            start=(hk == 0), stop=(hk == HK - 1),
                        )
                    nc.scalar.activation(sil[:, :csz], ps1[:, :csz], silu)
                    nc.vector.tensor_mul(
                        gt[fp][:, coff:coff + csz], sil[:, :csz], ps3[:, :csz]
                    )

            # Phase B: outT chunk [128 tokens, 1024] = sum_f gT.T @ w2T,
            # scaled by the per-token routing weight on eviction.  Runs
            # in the SAME PSUM pool tags as phase A (ps3 first: its
            # 4-back rotation slot is evicted well before the A tail) so
            # the PE rolls straight from the last A chain into B.
            for ck in range(CK):
                pb0 = ps.tile([P, 512], f32, tag="ps3", name=f"pb0_{ck}")
                pb1 = ps.tile([P, 512], f32, tag="ps1", name=f"pb1_{ck}")
                for fp in range(FP):
                    nc.tensor.matmul(
                        pb0[:], gt[fp][:, ck * P:(ck + 1) * P],
                        w2s[fp // W2B][:, (fp % W2B) * H:(fp % W2B) * H + 512],
                        start=(fp == 0), stop=(fp == FP - 1))
                o0 = ost.tile([P, 512], bf, tag="o0")
                nc.scalar.activation(o0[:], pb0[:], copy,
                                     scale=cvt[:, ck:ck + 1])
                nc.sync.dma_start(out=out[ck * P:(ck + 1) * P, 0:512],
                                  in_=o0[:])
                if ck < CK - 1:
                    for fp in range(FP):
                        nc.tensor.matmul(
                            pb1[:], gt[fp][:, ck * P:(ck + 1) * P],
                            w2s[fp // W2B][:, (fp % W2B) * H + 512:
                                           (fp % W2B) * H + 1024],
                            start=(fp == 0), stop=(fp == FP - 1))
                    o1 = ost.tile([P, 512], bf, tag="o1")
                    nc.vector.tensor_scalar_mul(o1[:], pb1[:],
                                                cvt[:, ck:ck + 1])
                    nc.scalar.dma_start(out=out[ck * P:(ck + 1) * P, 512:1024],
                                        in_=o1[:])
                else:
                    # Final half-block as two 256-col chains: the last
                    # eviction+DMA covers only 64KB, keeping the
                    # end-of-kernel drain ~1 us deep.
                    for qo in (0, 256):
                        for fp in range(FP):
                            nc.tensor.matmul(
                                pb1[:, qo:qo + 256],
                                gt[fp][:, ck * P:(ck + 1) * P],
                                w2s[fp // W2B][:, (fp % W2B) * H + 512 + qo:
                                               (fp % W2B) * H + 768 + qo],
                                start=(fp == 0), stop=(fp == FP - 1))
                        if qo == 0:
                            o1a = ost.tile([P, 256], bf, tag="o1a")
                            nc.scalar.activation(o1a[:], pb1[:, 0:256], copy,
                                                 scale=cvt[:, ck:ck + 1])
                            nc.scalar.dma_start(
                                out=out[ck * P:(ck + 1) * P, 512:768],
                                in_=o1a[:])
                        else:
                            o1b = ost.tile([P, 256], bf, tag="o1b")
                            nc.vector.tensor_scalar_mul(o1b[:], pb1[:, 256:512],
                                                        cvt[:, ck:ck + 1])
                            nc.sync.dma_start(
                                out=out[ck * P:(ck + 1) * P, 768:1024],
                                in_=o1b[:])

    nc.compile()
    return nc


def _silu(v):
    return v / (1.0 + np.exp(-v))


def kernel(hidden_states, gate_w, w1, w2, w3, _trace=False):
    global LAST_EXEC_TIME_NS
    _ensure_axon_hooks_stub()
    from concourse.bass_utils import run_bass_kernel_spmd

    x = np.asarray(hidden_states, dtype=np.float32).reshape(-1, H)
    gate_w = np.asarray(gate_w, dtype=np.float32)
    w1 = np.asarray(w1, dtype=np.float32)
    w2 = np.asarray(w2, dtype=np.float32)
    w3 = np.asarray(w3, dtype=np.float32)
    T = x.shape[0]

    # Router (f32, same math as the module): softmax over experts, top-2,
    # renormalized weights.
    logits = x @ gate_w.T
    p = np.exp(logits - logits.max(-1, keepdims=True))
    p /= p.sum(-1, keepdims=True)
    sel = np.argpartition(-p, TOP_K - 1, axis=-1)[:, :TOP_K]
    rw = np.take_along_axis(p, sel, axis=-1)
    rw = rw / rw.sum(-1, keepdims=True)

    idx_e, cv_e = [], []
    for e in range(E):
        hit = sel == e                      # [T, K]
        idx = np.nonzero(hit.any(axis=1))[0]
        w = np.where(hit[idx, 0], rw[idx, 0], rw[idx, 1])
        idx_e.append(idx)
        cv_e.append(w.astype(np.float32))

    if "nc" not in _BUILD_CACHE:
        _BUILD_CACHE["nc"] = _build()
    nc = _BUILD_CACHE["nc"]

    x_bf = x.astype(BF16)
    in_maps = []
    for e in range(E):
        idx = idx_e[e][:C]
        n = len(idx)
        xeT = np.zeros((H, C), dtype=BF16)
        xeT[:, :n] = x_bf[idx].T
        # [H, C] -> [P, HK*C]: partition p holds row hk*P+p of xeT at
        # columns [hk*C, (hk+1)*C) -- each xe slice DMA is fully linear.
        xep = np.ascontiguousarray(
            xeT.reshape(HK, P, C).transpose(1, 0, 2)).reshape(P, HK * C)
        cvp = np.ascontiguousarray(cv_e[e][:C].copy() if n == C else
                                   np.pad(cv_e[e][:n], (0, C - n))
                                   ).reshape(CK, P).T.astype(np.float32)
        cvp = np.ascontiguousarray(cvp)
        w1pk = np.ascontiguousarray(
            w1[e].astype(BF16).reshape(FP, P, HK, P).transpose(0, 3, 2, 1)
        ).reshape(FP, P, H)
        w3pk = np.ascontiguousarray(
            w3[e].astype(BF16).reshape(FP, P, HK, P).transpose(0, 3, 2, 1)
        ).reshape(FP, P, H)
        # Paired panel: cols [0,H) = w1, cols [H,2H) = w3 -- one DMA/fp.
        wpk = np.ascontiguousarray(np.concatenate([w1pk, w3pk], axis=2))
        w2pk = np.ascontiguousarray(w2[e].T.astype(BF16)).reshape(FP, P, H)
        w2fb = np.ascontiguousarray(
            w2pk.reshape(NW2, W2B, P, H).transpose(0, 2, 1, 3)
        ).reshape(NW2, P, W2B * H)
        in_maps.append({
            "xep": xep,
            "wpk": wpk,
            "w2f": w2fb,
            "cvp": cvp,
        })

    res = run_bass_kernel_spmd(
        nc, in_maps, core_ids=list(range(N_CORES)), trace=_trace
    )
    LAST_EXEC_TIME_NS = res.exec_time_ns

    out = np.zeros((T, H), dtype=np.float32)
    for e in range(E):
        idx = idx_e[e][:C]
        n = len(idx)
        if n:
            dev = np.asarray(res.results[e]["out"], dtype=np.float32)
            out[idx] += dev.reshape(C, H)[:n]
        # Capacity overflow (tokens beyond C for this expert): exact
        # host-side fp32 patch.  ~1.3% of pairs at balanced routing.
        ov = idx_e[e][C:]
        if len(ov):
            X = x[ov]
            h1 = X @ w1[e].T
            h3 = X @ w3[e].T
            g = _silu(h1) * h3 * cv_e[e][C:, None]
            out[ov] += g @ w2[e].T
    return out.reshape(B, S, H)


# revision 10
# speedup vs baseline: 1.0166x; 1.0166x over previous
"""Mixtral-style MoE (top-2 of 8 experts) on 8 TRN2 NeuronCores.

Strategy (expert-parallel, matching TENSOR_EXPERT_PARALLEL):
  - Host: router (logits -> softmax -> top-2 -> normalized weights), then
    shard: core e receives up to C=1024 tokens routed to expert e
    (gathered and pre-transposed to [H, C]) plus expert e's w1/w3/w2
    (bf16, pre-packed into PE-friendly [128 x free] tiles).  C=1024 is
    chosen so every core does identical, full-partition work (8 chunks
    of 128 tokens); the few overflow pairs beyond 1024 per expert
    (~1.3% of the 8192 token-expert pairs at balanced routing) are
    computed exactly on the host in fp32 and added into the output.
  - Device (SPMD, identical program on 8 cores): h1T = w1 @ xeT,
    h3T = w3 @ xeT, gT = silu(h1T) * h3T (bf16), outT = gT.T @ w2T,
    scaled per-token by the routing weight.  Pure GEMM pipeline; all
    DMAs are fully linear.
  - Host: scatter-add each core's [count_e, H] contribution into the
    [T, H] output (each token appears in exactly TOP_K=2 expert lists).

Compute is done in bf16 (fp32 accumulation in PSUM), which keeps the
TensorEngine at its 78.6 TF/s peak; sparse routing means each core does
C = 1024 token-columns instead of all 4096 (4x fewer FLOPs than dense).

v2 layout/startup notes (from baseline trace analysis):
  - The real-matmul stream is issue-dense at 216 ns per 512-col bf16
    matmul (the PE roofline); all remaining overhead is at the edges:
    ~12.3 us before the first real matmul (6.5 us fixed BSP preamble +
    DMA issue serialization + an 11x512-col warmup run at the HAM
    half-clock), ~1.5 us of PSUM-pool-transition stall between the
    up-proj and down-proj phases, and ~5.5 us of drain + teardown.
  - v2 therefore: (1) spreads the startup-critical DMAs across the
    Sync/GpSimd/Vector queues (Scalar starts with a compiler-hoisted
    1.3 us ACT_TABLE_LOAD, so it only gets non-critical loads),
    (2) uses short 128-col warmups that bridge only until the first
    real operands land (~1 us) letting the real stream ride the HAM
    ramp instead of waiting for it, (3) runs phase B in the SAME PSUM
    pool/tags as phase A so there is no pool barrier at the A->B
    transition, (4) fuses the per-panel DMAs (w1||w3 paired per fp,
    w2 in 4 blocks, xe in 5 slices of one [P, HK*C] tile, cv packed
    [P, CK]) cutting DMA issues from ~122 to ~58, and (5) finishes on
    two 256-col down-proj chains so the end-of-kernel drain is short.
"""

import numpy as np
import ml_dtypes

B, S, H, F, E, TOP_K = 2, 2048, 1024, 3584, 8, 2
N_CORES = 8
P = 128
HK = H // P   # 8 contraction chunks for up-proj
FP = F // P   # 28 partition chunks of the FFN dim
C = 1024      # per-core token capacity (8 full 128-token chunks)
CK = C // P
NW2 = 4       # w2 arrives in 4 fused blocks of 7 panels
W2B = FP // NW2
HQ = 2 * P    # startup-critical first columns of the fp0/fp1 panels
NWARM = 8     # 128-col HAM warmup matmuls (bridge until operands land)

BF16 = ml_dtypes.bfloat16

_BUILD_CACHE = {}
LAST_EXEC_TIME_NS = None


def _ensure_axon_hooks_stub():
    """bass_utils imports antenv.axon_hooks when BASS_TRACE is set; the
    agent image lacks it.  Register a None-hook stub so a stray
    BASS_TRACE env var degrades to an untraced run instead of crashing.
    """
    import sys, types

    try:
        import antenv.axon_hooks  # noqa: F401
        return
    except ImportError:
        pass
    mod = types.ModuleType("antenv.axon_hooks")
    mod._hook = None
    mod.set_axon_ntff_profile_hook = lambda h: setattr(mod, "_hook", h)
    mod.get_axon_ntff_profile_hook = lambda: mod._hook
    sys.modules["antenv.axon_hooks"] = mod
    try:
        import antenv

        antenv.axon_hooks = mod
    except ImportError:
        pass


def _build():
    """Build + compile the SPMD Bass program (token capacity C=1024)."""
    import concourse.bacc as bacc
    import concourse.mybir as mybir
    from concourse.tile import TileContext

    bf = mybir.dt.bfloat16
    f32 = mybir.dt.float32

    nc = bacc.Bacc("TRN2", target_bir_lowering=False, debug=False,
                   num_devices=N_CORES)
    xep = nc.dram_tensor("xep", [P, HK * C], bf, kind="ExternalInput")
    wpk = nc.dram_tensor("wpk", [FP, P, 2 * H], bf, kind="ExternalInput")
    w2f = nc.dram_tensor("w2f", [NW2, P, W2B * H], bf, kind="ExternalInput")
    cvp = nc.dram_tensor("cvp", [P, CK], f32, kind="ExternalInput")
    out = nc.dram_tensor("out", [C, H], bf, kind="ExternalOutput")

    cn_chunks = [(0, 512), (512, 512)]
    silu = mybir.ActivationFunctionType.Silu
    copy = mybir.ActivationFunctionType.Copy

    with TileContext(nc) as tc:
        with (
            tc.tile_pool(name="persist", bufs=1) as persist,
            tc.tile_pool(name="wload", bufs=3) as wload,
            tc.tile_pool(name="evac", bufs=4) as evac,
            tc.tile_pool(name="ost", bufs=3) as ost,
            tc.tile_pool(name="ps", bufs=4, space="PSUM") as ps,
        ):
            warm = persist.tile([P, 512], bf, tag="warm", name="warm")
            xet = persist.tile([P, HK * C], bf, tag="xe", name="xet")
            wt0 = persist.tile([P, 2 * H], bf, tag="wt0", name="wt0")
            wt1 = persist.tile([P, 2 * H], bf, tag="wt1", name="wt1")
            gt = [persist.tile([P, C], bf, tag=f"g{fp}", name=f"g{fp}")
                  for fp in range(FP)]
            w2s = [persist.tile([P, W2B * H], bf, tag=f"w2_{i}",
                                name=f"w2_{i}") for i in range(NW2)]
            cvt = persist.tile([P, CK], f32, tag="cv", name="cvt")

            # Startup: the warmup memset is Vector's first op; the
            # critical operand set (fp0/fp1 panel heads + xe chunk 0)
            # is spread over the Sync/GpSimd/Vector issue queues so the
            # first real matmul group unblocks ~1 us after the engines
            # come up.  Scalar opens with the compiler-hoisted
            # ACT_TABLE_LOAD (~1.3 us), so it only gets loads that are
            # not needed until the hk>=2 part of the lockstep.
            # GpSimd's SWDGE has low transfer bandwidth -- only the two
            # small w3 panel heads (+ tiny cv later) ride it.  All bulk
            # transfers go through the Sync/Scalar HWDGE queues; Sync
            # carries the startup-critical chain in consumption order.
            nc.vector.memset(warm[:], 0.0)
            nc.sync.dma_start(out=wt0[:, 0:HQ], in_=wpk[0][:, 0:HQ])
            nc.scalar.dma_start(out=wt0[:, H:H + HQ], in_=wpk[0][:, H:H + HQ])
            nc.sync.dma_start(out=xet[:, 0:512], in_=xep[:, 0:512])
            nc.sync.dma_start(out=wt1[:, 0:HQ], in_=wpk[1][:, 0:HQ])
            nc.scalar.dma_start(out=wt1[:, H:H + HQ], in_=wpk[1][:, H:H + HQ])
            nc.sync.dma_start(out=xet[:, 512:C], in_=xep[:, 512:C])
            nc.sync.dma_start(out=xet[:, C:2 * C], in_=xep[:, C:2 * C])
            nc.sync.dma_start(out=xet[:, 2 * C:3 * C], in_=xep[:, 2 * C:3 * C])
            nc.sync.dma_start(out=xet[:, 3 * C:HK * C], in_=xep[:, 3 * C:HK * C])
            nc.scalar.dma_start(out=wt0[:, HQ:H], in_=wpk[0][:, HQ:H])
            nc.scalar.dma_start(out=wt0[:, H + HQ:2 * H],
                                in_=wpk[0][:, H + HQ:2 * H])
            nc.scalar.dma_start(out=wt1[:, HQ:H], in_=wpk[1][:, HQ:H])
            nc.scalar.dma_start(out=wt1[:, H + HQ:2 * H],
                                in_=wpk[1][:, H + HQ:2 * H])

            # Short 128-col warmups: keep the PE busy (HAM ramp) for the
            # ~1 us until the first real operands land; the real stream
            # then continues the ramp doing useful work.
            wps = ps.tile([P, 512], f32, tag="ps1", name="wps")
            for i in range(NWARM):
                nc.tensor.matmul(wps[:, 0:P], warm[:, 0:P], warm[:, 0:P],
                                 start=True, stop=True)

            # Phase A: h1T/h3T = w1/w3 @ xeT per 128-row chunk of F,
            # fused SwiGLU into gT (bf16).  fp0 + fp1 run in hk-LOCKSTEP
            # so the startup DMA stream stays ahead of the PE.
            pss = {}
            for fp in (0, 1):
                for mat in (1, 3):
                    for ci in range(len(cn_chunks)):
                        pss[(fp, mat, ci)] = ps.tile(
                            [P, 512], f32, tag=f"ps{mat}",
                            name=f"ps{mat}_f{fp}_c{ci}",
                        )
            wts = {0: wt0, 1: wt1}
            for hk in range(HK):
                for ci, (coff, csz) in enumerate(cn_chunks):
                    for mat in (1, 3):
                        for fp in (0, 1):
                            off = 0 if mat == 1 else H
                            wt = wts[fp]
                            nc.tensor.matmul(
                                pss[(fp, mat, ci)][:, :csz],
                                wt[:, off + hk * P:off + (hk + 1) * P],
                                xet[:, hk * C + coff:hk * C + coff + csz],
                                start=(hk == 0), stop=(hk == HK - 1),
                            )
            for fp in (0, 1):
                for ci, (coff, csz) in enumerate(cn_chunks):
                    sil = evac.tile([P, 512], f32, tag="sil",
                                    name=f"sil_f{fp}_{ci}")
                    nc.scalar.activation(
                        sil[:, :csz], pss[(fp, 1, ci)][:, :csz], silu)
                    nc.vector.tensor_mul(
                        gt[fp][:, coff:coff + csz], sil[:, :csz],
                        pss[(fp, 3, ci)][:, :csz],
                    )
            for fp in range(2, FP):
                wt = wload.tile([P, 2 * H], bf, tag="wp")
                nc.sync.dma_start(out=wt[:], in_=wpk[fp])
                # Phase-B operands ride the idle GpSimd queue mid-phase-A
                # so their transfers overlap the A tail without delaying
                # the panel stream.
                if fp == 14:
                    nc.gpsimd.dma_start(out=cvt[:], in_=cvp[:, :])
                if fp in (16, 18, 20, 22):
                    i = (fp - 16) // 2
                    nc.scalar.dma_start(out=w2s[i][:], in_=w2f[i])
                for ci, (coff, csz) in enumerate(cn_chunks):
                    ps1 = ps.tile([P, 512], f32, tag="ps1")
                    ps3 = ps.tile([P, 512], f32, tag="ps3")
                    for hk in range(HK):
                        nc.tensor.matmul(
                            ps1[:, :csz],
                            wt[:, hk * P:(hk + 1) * P],
                            xet[:, hk * C + coff:hk * C + coff + csz],
                            start=(hk == 0), stop=(hk == HK - 1),
                        )
                    sil = evac.tile([P, 512], f32, tag="sil")
                    for hk in range(HK):
                        nc.tensor.matmul(
                            ps3[:, :csz],
                            wt[:, H + hk * P:H + (hk + 1) * P],
                            xet[:, hk * C + coff:hk * C + coff + csz],
                            start=(hk == 0), stop=(hk == HK - 1),
                        )
                    nc.scalar.activation(sil[:, :csz], ps1[:, :csz], silu)
                    nc.vector.tensor_mul(
                        gt[fp][:, coff:coff + csz], sil[:, :csz], ps3[:, :csz]
                    )

            # Phase B: outT chunk [128 tokens, 1024] = sum_f gT.T @ w2T,
            # scaled by the per-token routing weight on eviction.  Runs
            # in the SAME PSUM pool tags as phase A (ps3 first: its
            # 4-back rotation slot is evicted well before the A tail) so
            # the PE rolls straight from the last A chain into B.
            for ck in range(CK):
                pb0 = ps.tile([P, 512], f32, tag="ps3", name=f"pb0_{ck}")
                pb1 = ps.tile([P, 512], f32, tag="ps1", name=f"pb1_{ck}")
                for fp in range(FP):
                    nc.tensor.matmul(
                        pb0[:], gt[fp][:, ck * P:(ck + 1) * P],
                        w2s[fp // W2B][:, (fp % W2B) * H:(fp % W2B) * H + 512],
                        start=(fp == 0), stop=(fp == FP - 1))
                o0 = ost.tile([P, 512], bf, tag="o0")
                nc.scalar.activation(o0[:], pb0[:], copy,
                                     scale=cvt[:, ck:ck + 1])
                nc.sync.dma_start(out=out[ck * P:(ck + 1) * P, 0:512],
                                  in_=o0[:])
                if ck < CK - 1:
                    for fp in range(FP):
                        nc.tensor.matmul(
                            pb1[:], gt[fp][:, ck * P:(ck + 1) * P],
                            w2s[fp // W2B][:, (fp % W2B) * H + 512:
                                           (fp % W2B) * H + 1024],
                            start=(fp == 0), stop=(fp == FP - 1))
                    o1 = ost.tile([P, 512], bf, tag="o1")
                    nc.vector.tensor_scalar_mul(o1[:], pb1[:],
                                                cvt[:, ck:ck + 1])
                    nc.scalar.dma_start(out=out[ck * P:(ck + 1) * P, 512:1024],
                                        in_=o1[:])
                else:
                    # Final half-block as two 256-col chains in two
                    # DIFFERENT PSUM banks (same-bank halves would
                    # serialize the second chain behind the first
                    # half's eviction): the last eviction+DMA covers
                    # only 64KB, keeping the end-of-kernel drain short.
                    pb1b = ps.tile([P, 256], f32, tag="ps3", name="pb1b")
                    for fp in range(FP):
                        nc.tensor.matmul(
                            pb1[:, 0:256],
                            gt[fp][:, ck * P:(ck + 1) * P],
                            w2s[fp // W2B][:, (fp % W2B) * H + 512:
                                           (fp % W2B) * H + 768],
                            start=(fp == 0), stop=(fp == FP - 1))
                    for fp in range(FP):
                        nc.tensor.matmul(
                            pb1b[:],
                            gt[fp][:, ck * P:(ck + 1) * P],
                            w2s[fp // W2B][:, (fp % W2B) * H + 768:
                                           (fp % W2B) * H + 1024],
                            start=(fp == 0), stop=(fp == FP - 1))
                    o1a = ost.tile([P, 256], bf, tag="o1a")
                    nc.scalar.activation(o1a[:], pb1[:, 0:256], copy,
                                         scale=cvt[:, ck:ck + 1])
                    nc.scalar.dma_start(
                        out=out[ck * P:(ck + 1) * P, 512:768],
                        in_=o1a[:])
                    o1b = ost.tile([P, 256], bf, tag="o1b")
                    nc.vector.tensor_scalar_mul(o1b[:], pb1b[:],
                                                cvt[:, ck:ck + 1])
                    nc.sync.dma_start(
                        out=out[ck * P:(ck + 1) * P, 768:1024],
                        in_=o1b[:])

    nc.compile()
    return nc


def _silu(v):
    return v / (1.0 + np.exp(-v))


def kernel(hidden_states, gate_w, w1, w2, w3, _trace=False):
    global LAST_EXEC_TIME_NS
    _ensure_axon_hooks_stub()
    from concourse.bass_utils import run_bass_kernel_spmd

    x = np.asarray(hidden_states, dtype=np.float32).reshape(-1, H)
    gate_w = np.asarray(gate_w, dtype=np.float32)
    w1 = np.asarray(w1, dtype=np.float32)
    w2 = np.asarray(w2, dtype=np.float32)
    w3 = np.asarray(w3, dtype=np.float32)
    T = x.shape[0]

    # Router (f32, same math as the module): softmax over experts, top-2,
    # renormalized weights.
    logits = x @ gate_w.T
    p = np.exp(logits - logits.max(-1, keepdims=True))
    p /= p.sum(-1, keepdims=True)
    sel = np.argpartition(-p, TOP_K - 1, axis=-1)[:, :TOP_K]
    rw = np.take_along_axis(p, sel, axis=-1)
    rw = rw / rw.sum(-1, keepdims=True)

    idx_e, cv_e = [], []
    for e in range(E):
        hit = sel == e                      # [T, K]
        idx = np.nonzero(hit.any(axis=1))[0]
        w = np.where(hit[idx, 0], rw[idx, 0], rw[idx, 1])
        idx_e.append(idx)
        cv_e.append(w.astype(np.float32))

    if "nc" not in _BUILD_CACHE:
        _BUILD_CACHE["nc"] = _build()
    nc = _BUILD_CACHE["nc"]

    x_bf = x.astype(BF16)
    in_maps = []
    for e in range(E):
        idx = idx_e[e][:C]
        n = len(idx)
        xeT = np.zeros((H, C), dtype=BF16)
        xeT[:, :n] = x_bf[idx].T
        # [H, C] -> [P, HK*C]: partition p holds row hk*P+p of xeT at
        # columns [hk*C, (hk+1)*C) -- each xe slice DMA is fully linear.
        xep = np.ascontiguousarray(
            xeT.reshape(HK, P, C).transpose(1, 0, 2)).reshape(P, HK * C)
        cvp = np.ascontiguousarray(cv_e[e][:C].copy() if n == C else
                                   np.pad(cv_e[e][:n], (0, C - n))
                                   ).reshape(CK, P).T.astype(np.float32)
        cvp = np.ascontiguousarray(cvp)
        w1pk = np.ascontiguousarray(
            w1[e].astype(BF16).reshape(FP, P, HK, P).transpose(0, 3, 2, 1)
        ).reshape(FP, P, H)
        w3pk = np.ascontiguousarray(
            w3[e].astype(BF16).reshape(FP, P, HK, P).transpose(0, 3, 2, 1)
        ).reshape(FP, P, H)
        # Paired panel: cols [0,H) = w1, cols [H,2H) = w3 -- one DMA/fp.
        wpk = np.ascontiguousarray(np.concatenate([w1pk, w3pk], axis=2))
        w2pk = np.ascontiguousarray(w2[e].T.astype(BF16)).reshape(FP, P, H)
        w2fb = np.ascontiguousarray(
            w2pk.reshape(NW2, W2B, P, H).transpose(0, 2, 1, 3)
        ).reshape(NW2, P, W2B * H)
        in_maps.append({
            "xep": xep,
            "wpk": wpk,
            "w2f": w2fb,
            "cvp": cvp,
        })

    res = run_bass_kernel_spmd(
        nc, in_maps, core_ids=list(range(N_CORES)), trace=_trace
    )
    LAST_EXEC_TIME_NS = res.exec_time_ns

    out = np.zeros((T, H), dtype=np.float32)
    for e in range(E):
        idx = idx_e[e][:C]
        n = len(idx)
        if n:
            dev = np.asarray(res.results[e]["out"], dtype=np.float32)
            out[idx] += dev.reshape(C, H)[:n]
        # Capacity overflow (tokens beyond C for this expert): exact
        # host-side fp32 patch.  ~1.3% of pairs at balanced routing.
        ov = idx_e[e][C:]
        if len(ov):
            X = x[ov]
            h1 = X @ w1[e].T
            h3 = X @ w3[e].T
            g = _silu(h1) * h3 * cv_e[e][C:, None]
            out[ov] += g @ w2[e].T
    return out.reshape(B, S, H)
